# revision 1
# baseline (speedup 1.0000x reference)
"""Trainium2 Bass kernel for per-attribute MLP decoder (nn_AttrDecoder).

Computes, for each attribute a (A=312 independent blocks):
    h = relu(x[:, a*64:(a+1)*64] @ W1[a] + b1[a])      # [B, 128]
    o[:, a] = sigmoid(h @ W2[a] + b2[a])               # [B, 1]

Strategy:
  - Data-parallel over batch: B=8192 -> 1024 rows per core across 8 cores.
  - x is marshaled on the host to bf16 and transposed ([A*LAT, B]) so the
    contraction dim (LAT) lands on SBUF partitions as the tensor engine
    requires, and weights are pre-packed into PE-friendly layouts.
  - MM1: attributes in pairs; W1[2i] on PE rows 0-63, W1[2i+1] on rows
    64-127; two row-tiled K=64 matmuls run concurrently (tile_position
    (0,0)/(64,0)), N=512 batch columns, h^T accumulated in PSUM.
  - ReLU + b1 fused into the PSUM->SBUF copy (bf16 out), split across
    ScalarE (activation) and VectorE (tensor_scalar add+max) by throughput.
  - MM2: per attribute quad, four M=1 col-tiled matmuls (tile_position
    (0,32j), PSUM partitions {0,32,64,96} of a shared bank) so the four
    h-streams overlap on different PE column-groups.
  - Sigmoid + b2 on ScalarE per quad bank; strided-partition DMA stores
    rows 4q..4q+3 of the o^T output; host transposes during the gather.
"""

import numpy as np
import ml_dtypes

import concourse.bass as bass
import concourse.tile as tile
from concourse import mybir
from concourse import bass_utils

A = 312
LAT = 64
HID = 128
B = 8192
NCORES = 8
BS = B // NCORES          # 1024 batch rows per core
NPAIR = A // 2            # 156
NQUAD = A // 4            # 78
BT = 512                  # batch tile (one PSUM bank of fp32)
NBT = BS // BT            # 2

_cached = {}


def _legalize_waits(nc, max_waits=1):
    """Walrus in this toolchain encodes at most one sync-wait per instruction.
    Hoist extra waits onto standalone EventSemaphore instructions placed just
    before the owner on the same engine queue (queue order preserves the
    happens-before)."""
    nsplit = 0
    for bb in nc.m.functions[0].blocks:
        new_insts = []
        changed = False
        for inst in bb.instructions:
            si = getattr(inst, "sync_info", None)
            if si is not None and len(si.on_wait) > max_waits:
                waits = list(si.on_wait)
                for k, w in enumerate(waits[:-max_waits]):
                    es = mybir.InstEventSemaphore(name=f"{inst.name}-hw{k}")
                    es.engine = inst.engine
                    es.opcode = "EventSemaphore"
                    es.sync_info = mybir.SyncInfo(on_wait=[w], on_update=[])
                    new_insts.append(es)
                    nsplit += 1
                inst.sync_info = mybir.SyncInfo(
                    on_wait=waits[-max_waits:], on_update=list(si.on_update))
                changed = True
            new_insts.append(inst)
        if changed:
            bb.instructions = new_insts
    return nsplit


def _build_nc():
    nc = bass.Bass("TRN2", target_bir_lowering=False, debug=False,
                   num_devices=NCORES)
    xt = nc.dram_tensor("xt", [A * LAT, BS], mybir.dt.bfloat16,
                        kind="ExternalInput").ap()
    w1 = nc.dram_tensor("w1", [128, NPAIR, 128], mybir.dt.bfloat16,
                        kind="ExternalInput").ap()
    w2 = nc.dram_tensor("w2", [HID, A], mybir.dt.bfloat16,
                        kind="ExternalInput").ap()
    b1 = nc.dram_tensor("b1", [HID, A], mybir.dt.float32,
                        kind="ExternalInput").ap()
    b2 = nc.dram_tensor("b2", [128, NQUAD], mybir.dt.float32,
                        kind="ExternalInput").ap()
    ot = nc.dram_tensor("ot", [A, BS], mybir.dt.float32,
                        kind="ExternalOutput").ap()

    with tile.TileContext(nc, trace_sim=False) as tc:
        _body(tc, xt, w1, w2, b1, b2, ot)
    _legalize_waits(nc)
    return nc


def _body(tc, xt, w1, w2, b1, b2, ot):
    nc = tc.nc
    from contextlib import ExitStack
    with ExitStack() as ctx:
        singles = ctx.enter_context(tc.tile_pool(name="singles", bufs=1))
        xpool = ctx.enter_context(tc.tile_pool(name="x", bufs=6))
        hpool_a = ctx.enter_context(tc.tile_pool(name="ha", bufs=8))
        hpool_v = ctx.enter_context(tc.tile_pool(name="hv", bufs=8))
        opool = ctx.enter_context(tc.tile_pool(name="osb", bufs=4))
        hps = ctx.enter_context(
            tc.tile_pool(name="hps", bufs=3, space=bass.MemorySpace.PSUM))
        ops = ctx.enter_context(
            tc.tile_pool(name="ops", bufs=1, space=bass.MemorySpace.PSUM))

        # Resident weights/biases on the gpsimd SWDGE queue (~60 GB/s with
        # ~1us issue overhead per DMA, so order = arrival order). The v1
        # layout (all of b1/w2 before w1) stalled MM1 ~9us at pair 12
        # waiting for w1's second chunk; instead ship exactly what the
        # first pairs need first, then grow chunk sizes.
        b1_sb = singles.tile([HID, A], mybir.dt.float32)
        w2_sb = singles.tile([HID, A], mybir.dt.bfloat16)
        b2_sb = singles.tile([128, NQUAD], mybir.dt.float32)
        w1_sb = singles.tile([128, NPAIR, 128], mybir.dt.bfloat16)

        def w1_chunk(s, e):
            nc.gpsimd.dma_start(w1_sb[:, s:e, :], w1[:, s:e, :])

        # First pairs' weights ride the fast sync HWDGE queue ahead of x0
        # (the gpsimd SWDGE queue pays ~1us issue overhead per DMA, which
        # stacks up exactly when MM1 is starving); the bulk follows on
        # gpsimd so the x stream keeps the sync queue to itself.
        nc.sync.dma_start(w1_sb[:, 0:8, :], w1[:, 0:8, :])
        nc.gpsimd.dma_start(b1_sb[:, 0:64], b1[:, 0:64])
        nc.gpsimd.dma_start(w2_sb[:, 0:16], w2[:, 0:16])
        nc.gpsimd.dma_start(b2_sb[:], b2[:])
        w1_chunk(8, 16)
        nc.gpsimd.dma_start(b1_sb[:, 64:A], b1[:, 64:A])
        nc.gpsimd.dma_start(w2_sb[:, 16:A], w2[:, 16:A])
        for c, ce in [(16, 28), (28, 44), (44, 68), (68, 92), (92, 116),
                      (116, 140), (140, 156)]:
            w1_chunk(c, ce)

        def w1_slice(p, j):
            return w1_sb[j * 64:(j + 1) * 64, p, :]

        o_ps = ops.tile([128, NBT, BT], mybir.dt.float32, name="o_bank")

        def mm2_only(quad):
            """Emit one quad's 8 MM2s, bt-outer so the 4 attrs' streams hit
            4 different PE column-groups back-to-back (concurrent)."""
            for bt in range(NBT):
                for a, h_sb in quad:
                    jj = a % 4
                    nc.tensor.matmul(
                        o_ps[32 * jj:32 * jj + 1, bt, :],
                        w2_sb[:, a:a + 1],
                        h_sb[:, bt, :],
                        start=True, stop=True,
                        tile_position=(0, 32 * jj),
                    )

        def drain(q):
            """Sigmoid + store for quad q, emitted a full pair after its
            MM2s so the sigmoid never blocks the ACT queue on them."""
            o_sb = opool.tile([128, NBT, BT], mybir.dt.float32, name="osb")
            nc.scalar.activation(
                out=o_sb[:], in_=o_ps[:],
                func=mybir.ActivationFunctionType.Sigmoid,
                bias=b2_sb[:, q:q + 1], scale=1.0)
            nc.sync.dma_start(
                out=ot[4 * q:4 * q + 4, :].rearrange(
                    "p (n b) -> p n b", n=NBT),
                in_=o_sb[::32, :, :])

        pend = []           # (a, h_sb) entries not yet MM2'd
        sig_q = None        # quad whose sigmoid/store is deferred
        for p in range(NPAIR):
            x_tile = xpool.tile([128, BS], mybir.dt.bfloat16)
            nc.sync.dma_start(out=x_tile[:],
                              in_=xt[p * 128:(p + 1) * 128, :])
            h_pss = [hps.tile([128, NBT, BT], mybir.dt.float32, name="hps"),
                     hps.tile([128, NBT, BT], mybir.dt.float32, name="hps")]
            # MM1s: interleave the two attrs (disjoint PE row groups) so
            # their streams overlap; bt-halves of one attr are sequential.
            for bt in range(NBT):
                for j in range(2):
                    nc.tensor.matmul(
                        h_pss[j][:, bt, :],
                        w1_slice(p, j),
                        x_tile[j * 64:(j + 1) * 64, bass.ds(bt * BT, BT)],
                        start=True, stop=True,
                        tile_position=(j * 64, 0),
                    )
            # relu: one FD=1024 op per attr, alternating engines
            new_pend = []
            for j in range(2):
                a = 2 * p + j
                use_act = (a % 2 == 0) and (a % 12 != 0)
                hp = hpool_a if use_act else hpool_v
                h_sb = hp.tile([HID, NBT, BT], mybir.dt.bfloat16,
                               name="hsb")
                if use_act:
                    nc.scalar.activation(
                        out=h_sb[:], in_=h_pss[j][:],
                        func=mybir.ActivationFunctionType.Relu,
                        bias=b1_sb[:, a:a + 1], scale=1.0)
                else:
                    nc.vector.tensor_scalar(
                        out=h_sb[:], in0=h_pss[j][:],
                        scalar1=b1_sb[:, a:a + 1], scalar2=0.0,
                        op0=mybir.AluOpType.add,
                        op1=mybir.AluOpType.max)
                new_pend.append((a, h_sb))
            # MM2s for the oldest complete quad (its relus had 1-2 pairs
            # of MM1 streams to complete behind); sigmoid one cycle later
            if len(pend) >= 4:
                if sig_q is not None:
                    drain(sig_q)
                mm2_only(pend[:4])
                sig_q = pend[0][0] // 4
                pend = pend[4:]
            pend += new_pend
        while pend:
            if sig_q is not None:
                drain(sig_q)
            mm2_only(pend[:4])
            sig_q = pend[0][0] // 4
            pend = pend[4:]
        drain(sig_q)


def _install_ntff_hook():
    """Register the axon NTFF profile hook (normally provided by the agent
    image's antenv.axon_hooks). Needed only for trace=True runs."""
    import sys as _sys, types as _types, ctypes, contextlib

    if "antenv.axon_hooks" not in _sys.modules:
        mod = _types.ModuleType("antenv.axon_hooks")
        _h = [None]
        mod.set_axon_ntff_profile_hook = lambda h: _h.__setitem__(0, h)
        mod.get_axon_ntff_profile_hook = lambda: _h[0]
        _sys.modules["antenv.axon_hooks"] = mod
        try:
            import antenv
            antenv.axon_hooks = mod
        except ImportError:
            pass
    mod = _sys.modules["antenv.axon_hooks"]
    if mod.get_axon_ntff_profile_hook() is not None:
        return

    lib = ctypes.CDLL("/opt/axon/libaxon_pjrt.so")
    lib.axon_start_nrt_profile.argtypes = [
        ctypes.POINTER(ctypes.c_int64), ctypes.c_size_t]
    lib.axon_start_nrt_profile.restype = ctypes.c_int64
    lib.axon_stop_nrt_profile.argtypes = [ctypes.c_char_p]
    lib.axon_stop_nrt_profile.restype = ctypes.c_int64

    @contextlib.contextmanager
    def _hook(output_dir, device_ids):
        import jax
        jax.devices()
        if device_ids:
            ids = (ctypes.c_int64 * len(device_ids))(*device_ids)
            rc = lib.axon_start_nrt_profile(ids, len(device_ids))
        else:
            rc = lib.axon_start_nrt_profile(None, 0)
        if rc != 0:
            raise RuntimeError(f"axon_start_nrt_profile rc={rc}")
        try:
            yield
        finally:
            n = lib.axon_stop_nrt_profile(str(output_dir).encode())
            print(f"ntff profile: {n} file(s) -> {output_dir}")

    mod.set_axon_ntff_profile_hook(_hook)
    # artifact upload needs a bucket; stub it out for local profiling
    bass_utils.upload_artifacts = lambda tmpdir: f"local://{tmpdir}"


def kernel(x, W1, b1, W2, b2, trace=False):
    if "nc" not in _cached:
        _cached["nc"] = _build_nc()
    nc = _cached["nc"]
    if trace:
        try:
            _install_ntff_hook()
        except Exception as e:
            print("ntff hook install failed:", e)
            trace = False

    xt = np.ascontiguousarray(
        x.reshape(B, A * LAT).astype(ml_dtypes.bfloat16).T)     # [19968, 8192]
    w1h = np.ascontiguousarray(
        W1.reshape(NPAIR, 128, 128).transpose(1, 0, 2)).astype(
            ml_dtypes.bfloat16)                                  # [128,156,128]
    w2h = np.ascontiguousarray(
        W2.reshape(A, HID).T).astype(ml_dtypes.bfloat16)         # [128, 312]
    b1h = np.ascontiguousarray(b1.T).astype(np.float32)          # [128, 312]
    b2h = np.zeros((128, NQUAD), np.float32)
    b2h[::32, :] = b2.reshape(NQUAD, 4).T

    in_maps = []
    for c in range(NCORES):
        in_maps.append({
            "xt": np.ascontiguousarray(xt[:, c * BS:(c + 1) * BS]),
            "w1": w1h, "w2": w2h, "b1": b1h, "b2": b2h,
        })

    res = bass_utils.run_bass_kernel_spmd(
        nc, in_maps, core_ids=list(range(NCORES)), trace=trace)
    _cached["last_results"] = res

    out = np.empty((B, A), np.float32)
    for c in range(NCORES):
        out[c * BS:(c + 1) * BS, :] = res.results[c]["ot"].T
    return out



# revision 6
# speedup vs baseline: 1.0332x; 1.0332x over previous
"""Trainium2 Bass kernel for per-attribute MLP decoder (nn_AttrDecoder).

Computes, for each attribute a (A=312 independent blocks):
    h = relu(x[:, a*64:(a+1)*64] @ W1[a] + b1[a])      # [B, 128]
    o[:, a] = sigmoid(h @ W2[a] + b2[a])               # [B, 1]

v2 design notes (vs the 292us v1 baseline):
  - The binding constraint is the PSUM->SBUF relu pass over h (40.9M
    elems/core): only ACT and DVE can read PSUM, at ~1 elem/cycle for
    fp32 sources, so that stage floors at ~170us across both engines.
    Everything else is arranged to stay under that and off those engines.
  - v1 burned ~86us of ACT on sigmoids over [128,1024] tiles with only
    4 live partitions (one per PE column-group).  v2 packs o densely:
    MM2 for attr a uses an M=32 one-hot stationary (W2[a] in column
    (a%128)//4, zeros elsewhere) accumulated onto a shared PSUM bank at
    tile_position (0, 32*(a%4)), so 128 attrs land on 128 distinct
    partitions.  Sigmoid+b2 then runs once per 128-attr group (3 ops
    total, ~3us) with b2 as a per-partition bias, and the store is one
    dense DMA per group (host undoes the partition permutation).
  - x DMAs are 3 MB each (12 pairs, 24 KB contiguous per partition)
    instead of 156x 256 KB: near line-rate HBM streaming.
  - relu work is split ACT:DVE ~169:143 to balance their effective
    per-op rates (1.01us vs 1.19us for FD=1024).
"""

import numpy as np
import ml_dtypes

import concourse.bass as bass
import concourse.tile as tile
from concourse import mybir
from concourse import bass_utils

A = 312
LAT = 64
HID = 128
B = 8192
NCORES = 8
BS = B // NCORES          # 1024 batch rows per core
NPAIR = A // 2            # 156
NQUAD = A // 4            # 78
BT = 512                  # batch tile (one PSUM bank of fp32)
NBT = BS // BT            # 2
XBLK = 12                 # pairs per x DMA (3 MB, 24 KB/partition)
NGRP = (A + 127) // 128   # 3 output groups (128 attrs dense per group)
N_ACT = 169               # relu ops assigned to ScalarE (rest on DVE)

_cached = {}


def _legalize_waits(nc, max_waits=1):
    """Walrus in this toolchain encodes at most one sync-wait per instruction.
    Hoist extra waits onto standalone EventSemaphore instructions placed just
    before the owner on the same engine queue (queue order preserves the
    happens-before)."""
    nsplit = 0
    for bb in nc.m.functions[0].blocks:
        new_insts = []
        changed = False
        for inst in bb.instructions:
            si = getattr(inst, "sync_info", None)
            if si is not None and len(si.on_wait) > max_waits:
                waits = list(si.on_wait)
                for k, w in enumerate(waits[:-max_waits]):
                    es = mybir.InstEventSemaphore(name=f"{inst.name}-hw{k}")
                    es.engine = inst.engine
                    es.opcode = "EventSemaphore"
                    es.sync_info = mybir.SyncInfo(on_wait=[w], on_update=[])
                    new_insts.append(es)
                    nsplit += 1
                inst.sync_info = mybir.SyncInfo(
                    on_wait=waits[-max_waits:], on_update=list(si.on_update))
                changed = True
            new_insts.append(inst)
        if changed:
            bb.instructions = new_insts
    return nsplit


def _build_nc():
    nc = bass.Bass("TRN2", target_bir_lowering=False, debug=False,
                   num_devices=NCORES)
    # xs[r, q, :] = x^T[q*128 + r, :] so each pair q is one partition-dim
    # slice and per-partition reads are contiguous across consecutive pairs.
    xs = nc.dram_tensor("xs", [128, NPAIR, BS], mybir.dt.bfloat16,
                        kind="ExternalInput").ap()
    w1 = nc.dram_tensor("w1", [128, NPAIR, 128], mybir.dt.bfloat16,
                        kind="ExternalInput").ap()
    # One-hot padded W2: w2oh[:, a, i] = W2[a] if i == (a%128)//4 else 0.
    w2 = nc.dram_tensor("w2", [HID, A, 32], mybir.dt.bfloat16,
                        kind="ExternalInput").ap()
    b1 = nc.dram_tensor("b1", [HID, A], mybir.dt.float32,
                        kind="ExternalInput").ap()
    # b2 permuted to the dense-o partition order: b2g[p, g] = b2[g*128+m]
    # with p = 32*(m%4) + m//4.
    b2 = nc.dram_tensor("b2", [128, NGRP], mybir.dt.float32,
                        kind="ExternalInput").ap()
    ot = nc.dram_tensor("ot", [NGRP * 128, BS], mybir.dt.float32,
                        kind="ExternalOutput").ap()

    with tile.TileContext(nc, trace_sim=False) as tc:
        _body(tc, xs, w1, w2, b1, b2, ot)
    _legalize_waits(nc)
    return nc


def _body(tc, xs, w1, w2, b1, b2, ot):
    nc = tc.nc
    from contextlib import ExitStack
    with ExitStack() as ctx:
        singles = ctx.enter_context(tc.tile_pool(name="singles", bufs=1))
        xpool = ctx.enter_context(tc.tile_pool(name="x", bufs=2))
        hsb = ctx.enter_context(tc.tile_pool(name="hsb", bufs=10))
        osb = ctx.enter_context(tc.tile_pool(name="osb", bufs=2))
        hps = ctx.enter_context(
            tc.tile_pool(name="hps", bufs=3, space=bass.MemorySpace.PSUM))
        ops = ctx.enter_context(
            tc.tile_pool(name="ops", bufs=1, space=bass.MemorySpace.PSUM))

        b1_sb = singles.tile([HID, A], mybir.dt.float32)
        w2_sb = singles.tile([HID, A, 32], mybir.dt.bfloat16)
        b2_sb = singles.tile([128, NGRP], mybir.dt.float32)
        w1_sb = singles.tile([128, NPAIR, 128], mybir.dt.bfloat16)

        # Resident weights: ship what the first pairs need on the fast sync
        # HWDGE queue, bulk follows on the gpsimd SWDGE queue in growing
        # chunks (issue overhead ~1us per SWDGE dma).
        nc.sync.dma_start(w1_sb[:, 0:8, :], w1[:, 0:8, :])
        nc.gpsimd.dma_start(b1_sb[:, 0:64], b1[:, 0:64])
        nc.gpsimd.dma_start(w2_sb[:, 0:32, :], w2[:, 0:32, :])
        nc.gpsimd.dma_start(b2_sb[:], b2[:])
        nc.gpsimd.dma_start(w1_sb[:, 8:20, :], w1[:, 8:20, :])
        nc.gpsimd.dma_start(b1_sb[:, 64:A], b1[:, 64:A])
        nc.gpsimd.dma_start(w2_sb[:, 32:128, :], w2[:, 32:128, :])
        for c, ce in [(20, 36), (36, 60), (60, 90), (90, 124), (124, 156)]:
            nc.gpsimd.dma_start(w1_sb[:, c:ce, :], w1[:, c:ce, :])
        nc.gpsimd.dma_start(w2_sb[:, 128:A, :], w2[:, 128:A, :])

        # relu engine assignment: N_ACT attrs on ScalarE, rest on DVE,
        # interleaved evenly.
        use_act = [((a + 1) * N_ACT) // A > (a * N_ACT) // A
                   for a in range(A)]

        o_ps = [None]       # current group's dense o accumulator

        def mm2_quad(k, quad):
            """Quad k = attrs 4k..4k+3 -> col-groups 0..3 (concurrent
            streams), accumulating into o_ps at chain position i=k%32."""
            i = k % 32
            stop = (i == 31) or (k == NQUAD - 1)
            if i == 0:
                o_ps[0] = ops.tile([128, NBT, BT], mybir.dt.float32,
                                   name="o_dense")
            for bt in range(NBT):
                for t, (a, h_sb) in enumerate(quad):
                    # start=True clears has_written for the written region
                    # only (verified on HW: bank-wide-clear would break the
                    # other col-groups' chains), so each chain opens with it.
                    nc.tensor.matmul(
                        o_ps[0][32 * t:32 * t + 32, bt, :],
                        w2_sb[:, a, :],
                        h_sb[:, bt, :],
                        start=(i == 0), stop=stop,
                        tile_position=(0, 32 * t),
                    )

        def sigmoid_store(g):
            """Dense sigmoid + store for 128-attr group g."""
            o_out = osb.tile([128, NBT, BT], mybir.dt.float32, name="osb")
            nc.scalar.activation(
                out=o_out[:], in_=o_ps[0][:],
                func=mybir.ActivationFunctionType.Sigmoid,
                bias=b2_sb[:, g:g + 1], scale=1.0)
            nc.sync.dma_start(
                out=ot[g * 128:(g + 1) * 128, :].rearrange(
                    "p (n b) -> p n b", n=NBT),
                in_=o_out[:])

        x_tile = [None]
        pend = []           # (a, h_sb) relu'd attrs not yet MM2'd
        sig_g = None        # group whose sigmoid/store is deferred
        next_quad = 0
        for p in range(NPAIR):
            blk, off = divmod(p, XBLK)
            if off == 0:
                x_tile[0] = xpool.tile([128, min(XBLK, NPAIR - p), BS],
                                       mybir.dt.bfloat16, name="xt")
                nc.sync.dma_start(
                    out=x_tile[0][:],
                    in_=xs[:, p:p + x_tile[0].shape[1], :])
            # Emit the deferred sigmoid and the oldest complete quad's MM2s
            # BEFORE this pair's MM1s: the MM1 that waits on a PSUM slot
            # (3-deep h pool) must not block ready MM2 work behind it on
            # the strict-FIFO PE queue.
            if len(pend) >= 6:
                if sig_g is not None:
                    sigmoid_store(sig_g)
                    sig_g = None
                mm2_quad(next_quad, pend[:4])
                if (next_quad % 32 == 31) or next_quad == NQUAD - 1:
                    sig_g = next_quad // 32
                next_quad += 1
                pend = pend[4:]
            h_pss = [hps.tile([128, NBT, BT], mybir.dt.float32, name="hps"),
                     hps.tile([128, NBT, BT], mybir.dt.float32, name="hps")]
            # j-outer: j0's two matmuls stream while j1 may still wait on
            # its slot's relu (WAR); bt-outer would trap j0's bt1 behind
            # j1's stall.
            for j in range(2):
                for bt in range(NBT):
                    nc.tensor.matmul(
                        h_pss[j][:, bt, :],
                        w1_sb[j * 64:(j + 1) * 64, p, :],
                        x_tile[0][j * 64:(j + 1) * 64, off,
                                  bass.ds(bt * BT, BT)],
                        start=True, stop=True,
                        tile_position=(j * 64, 0),
                    )
            for j in range(2):
                a = 2 * p + j
                h_sb = hsb.tile([HID, NBT, BT], mybir.dt.bfloat16,
                                name="hsb")
                if use_act[a]:
                    nc.scalar.activation(
                        out=h_sb[:], in_=h_pss[j][:],
                        func=mybir.ActivationFunctionType.Relu,
                        bias=b1_sb[:, a:a + 1], scale=1.0)
                else:
                    nc.vector.tensor_scalar(
                        out=h_sb[:], in0=h_pss[j][:],
                        scalar1=b1_sb[:, a:a + 1], scalar2=0.0,
                        op0=mybir.AluOpType.add,
                        op1=mybir.AluOpType.max)
                pend.append((a, h_sb))
            # Emit the deferred sigmoid a couple of pairs after its last MM2
            # so it never blocks the ACT queue head, then the oldest
            # complete quad's MM2s (their relus have had ~2 pairs to drain).
            if len(pend) >= 8:
                if sig_g is not None:
                    sigmoid_store(sig_g)
                    sig_g = None
                mm2_quad(next_quad, pend[:4])
                if (next_quad % 32 == 31) or next_quad == NQUAD - 1:
                    sig_g = next_quad // 32
                next_quad += 1
                pend = pend[4:]
        while pend:
            if sig_g is not None:
                sigmoid_store(sig_g)
                sig_g = None
            mm2_quad(next_quad, pend[:4])
            if (next_quad % 32 == 31) or next_quad == NQUAD - 1:
                sig_g = next_quad // 32
            next_quad += 1
            pend = pend[4:]
        sigmoid_store(sig_g)


def _install_ntff_hook():
    """Register the axon NTFF profile hook (normally provided by the agent
    image's antenv.axon_hooks). Needed only for trace=True runs."""
    import sys as _sys, types as _types, ctypes, contextlib

    if "antenv.axon_hooks" not in _sys.modules:
        mod = _types.ModuleType("antenv.axon_hooks")
        _h = [None]
        mod.set_axon_ntff_profile_hook = lambda h: _h.__setitem__(0, h)
        mod.get_axon_ntff_profile_hook = lambda: _h[0]
        _sys.modules["antenv.axon_hooks"] = mod
        try:
            import antenv
            antenv.axon_hooks = mod
        except ImportError:
            pass
    mod = _sys.modules["antenv.axon_hooks"]
    if mod.get_axon_ntff_profile_hook() is not None:
        return

    lib = ctypes.CDLL("/opt/axon/libaxon_pjrt.so")
    lib.axon_start_nrt_profile.argtypes = [
        ctypes.POINTER(ctypes.c_int64), ctypes.c_size_t]
    lib.axon_start_nrt_profile.restype = ctypes.c_int64
    lib.axon_stop_nrt_profile.argtypes = [ctypes.c_char_p]
    lib.axon_stop_nrt_profile.restype = ctypes.c_int64

    @contextlib.contextmanager
    def _hook(output_dir, device_ids):
        import jax
        jax.devices()
        if device_ids:
            ids = (ctypes.c_int64 * len(device_ids))(*device_ids)
            rc = lib.axon_start_nrt_profile(ids, len(device_ids))
        else:
            rc = lib.axon_start_nrt_profile(None, 0)
        if rc != 0:
            raise RuntimeError(f"axon_start_nrt_profile rc={rc}")
        try:
            yield
        finally:
            n = lib.axon_stop_nrt_profile(str(output_dir).encode())
            print(f"ntff profile: {n} file(s) -> {output_dir}")

    mod.set_axon_ntff_profile_hook(_hook)
    # artifact upload needs a bucket; stub it out for local profiling
    bass_utils.upload_artifacts = lambda tmpdir: f"local://{tmpdir}"


def kernel(x, W1, b1, W2, b2, trace=False):
    if "nc" not in _cached:
        _cached["nc"] = _build_nc()
    nc = _cached["nc"]
    if trace:
        try:
            _install_ntff_hook()
        except Exception as e:
            print("ntff hook install failed:", e)
            trace = False

    xt = np.ascontiguousarray(
        x.reshape(B, A * LAT).astype(ml_dtypes.bfloat16).T)     # [19968, 8192]
    w1h = np.ascontiguousarray(
        W1.reshape(NPAIR, 128, 128).transpose(1, 0, 2)).astype(
            ml_dtypes.bfloat16)                                  # [128,156,128]
    # one-hot padded W2: column (a%128)//4 of slot a holds W2[a]
    w2h = np.zeros((HID, A, 32), np.float32)
    cols = (np.arange(A) % 128) // 4
    w2h[:, np.arange(A), cols] = W2.reshape(A, HID).T
    w2h = w2h.astype(ml_dtypes.bfloat16)
    b1h = np.ascontiguousarray(b1.T).astype(np.float32)          # [128, 312]
    # b2 permuted to dense-o partition order
    b2h = np.zeros((128, NGRP), np.float32)
    for g in range(NGRP):
        m = np.arange(min(128, A - g * 128))
        b2h[32 * (m % 4) + m // 4, g] = b2.reshape(A)[g * 128 + m]

    in_maps = []
    for c in range(NCORES):
        xc = xt[:, c * BS:(c + 1) * BS]                          # [19968,1024]
        xsh = np.ascontiguousarray(
            xc.reshape(NPAIR, 128, BS).transpose(1, 0, 2))       # [128,156,1024]
        in_maps.append({
            "xs": xsh, "w1": w1h, "w2": w2h, "b1": b1h, "b2": b2h,
        })

    res = bass_utils.run_bass_kernel_spmd(
        nc, in_maps, core_ids=list(range(NCORES)), trace=trace)
    _cached["last_results"] = res

    # undo the dense-o partition permutation: attr a = g*128+m lives at
    # device row g*128 + 32*(m%4) + m//4
    aa = np.arange(A)
    g, m = aa // 128, aa % 128
    rows = g * 128 + 32 * (m % 4) + m // 4
    out = np.empty((B, A), np.float32)
    for c in range(NCORES):
        out[c * BS:(c + 1) * BS, :] = res.results[c]["ot"][rows, :].T
    return out


# revision 8
# speedup vs baseline: 1.0822x; 1.0474x over previous
"""Trainium2 Bass kernel for per-attribute MLP decoder (nn_AttrDecoder).

Computes, for each attribute a (A=312 independent blocks):
    h = relu(x[:, a*64:(a+1)*64] @ W1[a] + b1[a])      # [B, 128]
    o[:, a] = sigmoid(h @ W2[a] + b2[a])               # [B, 1]

v2 design notes (vs the 292us v1 baseline):
  - The binding constraint is the PSUM->SBUF relu pass over h (40.9M
    elems/core): only ACT and DVE can read PSUM, at ~1 elem/cycle for
    fp32 sources, so that stage floors at ~170us across both engines.
    Everything else is arranged to stay under that and off those engines.
  - v1 burned ~86us of ACT on sigmoids over [128,1024] tiles with only
    4 live partitions (one per PE column-group).  v2 packs o densely:
    MM2 for attr a uses an M=32 one-hot stationary (W2[a] in column
    (a%128)//4, zeros elsewhere) accumulated onto a shared PSUM bank at
    tile_position (0, 32*(a%4)), so 128 attrs land on 128 distinct
    partitions.  Sigmoid+b2 then runs once per 128-attr group (3 ops
    total, ~3us) with b2 as a per-partition bias, and the store is one
    dense DMA per group (host undoes the partition permutation).
  - x DMAs are 3 MB each (12 pairs, 24 KB contiguous per partition)
    instead of 156x 256 KB: near line-rate HBM streaming.
  - relu work is split ACT:DVE ~169:143 to balance their effective
    per-op rates (1.01us vs 1.19us for FD=1024).
"""

import numpy as np
import ml_dtypes

import concourse.bass as bass
import concourse.tile as tile
from concourse import mybir
from concourse import bass_utils

A = 312
LAT = 64
HID = 128
B = 8192
NCORES = 8
BS = B // NCORES          # 1024 batch rows per core
NPAIR = A // 2            # 156
NQUAD = A // 4            # 78
BT = 512                  # batch tile (one PSUM bank of fp32)
NBT = BS // BT            # 2
XBLK = 12                 # pairs per x DMA (3 MB, 24 KB/partition)
NGRP = (A + 127) // 128   # 3 output groups (128 attrs dense per group)
N_ACT = 167               # relu ops assigned to ScalarE (rest on DVE)

_cached = {}


def _legalize_waits(nc, max_waits=1):
    """Walrus in this toolchain encodes at most one sync-wait per instruction.
    Hoist extra waits onto standalone EventSemaphore instructions placed just
    before the owner on the same engine queue (queue order preserves the
    happens-before)."""
    nsplit = 0
    for bb in nc.m.functions[0].blocks:
        new_insts = []
        changed = False
        for inst in bb.instructions:
            si = getattr(inst, "sync_info", None)
            if si is not None and len(si.on_wait) > max_waits:
                waits = list(si.on_wait)
                for k, w in enumerate(waits[:-max_waits]):
                    es = mybir.InstEventSemaphore(name=f"{inst.name}-hw{k}")
                    es.engine = inst.engine
                    es.opcode = "EventSemaphore"
                    es.sync_info = mybir.SyncInfo(on_wait=[w], on_update=[])
                    new_insts.append(es)
                    nsplit += 1
                inst.sync_info = mybir.SyncInfo(
                    on_wait=waits[-max_waits:], on_update=list(si.on_update))
                changed = True
            new_insts.append(inst)
        if changed:
            bb.instructions = new_insts
    return nsplit


def _build_nc():
    nc = bass.Bass("TRN2", target_bir_lowering=False, debug=False,
                   num_devices=NCORES)
    # xs[r, q, :] = x^T[q*128 + r, :] so each pair q is one partition-dim
    # slice and per-partition reads are contiguous across consecutive pairs.
    xs = nc.dram_tensor("xs", [128, NPAIR, BS], mybir.dt.bfloat16,
                        kind="ExternalInput").ap()
    w1 = nc.dram_tensor("w1", [128, NPAIR, 128], mybir.dt.bfloat16,
                        kind="ExternalInput").ap()
    # One-hot padded W2: w2oh[:, a, i] = W2[a] if i == (a%128)//4 else 0.
    w2 = nc.dram_tensor("w2", [HID, A, 32], mybir.dt.bfloat16,
                        kind="ExternalInput").ap()
    b1 = nc.dram_tensor("b1", [HID, A], mybir.dt.float32,
                        kind="ExternalInput").ap()
    # b2 permuted to the dense-o partition order: b2g[p, g] = b2[g*128+m]
    # with p = 32*(m%4) + m//4.
    b2 = nc.dram_tensor("b2", [128, NGRP], mybir.dt.float32,
                        kind="ExternalInput").ap()
    ot = nc.dram_tensor("ot", [NGRP * 128, BS], mybir.dt.float32,
                        kind="ExternalOutput").ap()

    with tile.TileContext(nc, trace_sim=False) as tc:
        _body(tc, xs, w1, w2, b1, b2, ot)
    _legalize_waits(nc)
    return nc


def _body(tc, xs, w1, w2, b1, b2, ot):
    nc = tc.nc
    from contextlib import ExitStack
    with ExitStack() as ctx:
        singles = ctx.enter_context(tc.tile_pool(name="singles", bufs=1))
        xpool = ctx.enter_context(tc.tile_pool(name="x", bufs=2))
        hsb = ctx.enter_context(tc.tile_pool(name="hsb", bufs=10))
        osb = ctx.enter_context(tc.tile_pool(name="osb", bufs=2))
        hps = ctx.enter_context(
            tc.tile_pool(name="hps", bufs=3, space=bass.MemorySpace.PSUM))
        ops = ctx.enter_context(
            tc.tile_pool(name="ops", bufs=1, space=bass.MemorySpace.PSUM))

        b1_sb = singles.tile([HID, A], mybir.dt.float32)
        w2_sb = singles.tile([HID, A, 32], mybir.dt.bfloat16)
        b2_sb = singles.tile([128, NGRP], mybir.dt.float32)
        w1_sb = singles.tile([128, NPAIR, 128], mybir.dt.bfloat16)

        # Resident weights: ship what the first pairs need on the fast sync
        # HWDGE queue, bulk follows on the gpsimd SWDGE queue in growing
        # chunks (issue overhead ~1us per SWDGE dma).
        nc.sync.dma_start(w1_sb[:, 0:8, :], w1[:, 0:8, :])
        nc.gpsimd.dma_start(b1_sb[:, 0:64], b1[:, 0:64])
        nc.gpsimd.dma_start(w2_sb[:, 0:32, :], w2[:, 0:32, :])
        nc.gpsimd.dma_start(b2_sb[:], b2[:])
        nc.gpsimd.dma_start(w1_sb[:, 8:20, :], w1[:, 8:20, :])
        nc.gpsimd.dma_start(b1_sb[:, 64:A], b1[:, 64:A])
        nc.gpsimd.dma_start(w2_sb[:, 32:128, :], w2[:, 32:128, :])
        for c, ce in [(20, 36), (36, 60), (60, 90), (90, 124), (124, 156)]:
            nc.gpsimd.dma_start(w1_sb[:, c:ce, :], w1[:, c:ce, :])
        nc.gpsimd.dma_start(w2_sb[:, 128:A, :], w2[:, 128:A, :])

        # relu engine assignment: N_ACT attrs on ScalarE, rest on DVE,
        # interleaved evenly.
        use_act = [((a + 1) * N_ACT) // A > (a * N_ACT) // A
                   for a in range(A)]

        o_ps = [None]       # current group's dense o accumulator

        def mm2_quad(k, quad):
            """Quad k = attrs 4k..4k+3 -> col-groups 0..3 (concurrent
            streams), accumulating into o_ps at chain position i=k%32."""
            i = k % 32
            stop = (i == 31) or (k == NQUAD - 1)
            if i == 0:
                o_ps[0] = ops.tile([128, NBT, BT], mybir.dt.float32,
                                   name="o_dense")
            for bt in range(NBT):
                for t, (a, h_sb) in enumerate(quad):
                    # start=True clears has_written for the written region
                    # only (verified on HW: bank-wide-clear would break the
                    # other col-groups' chains), so each chain opens with it.
                    nc.tensor.matmul(
                        o_ps[0][32 * t:32 * t + 32, bt, :],
                        w2_sb[:, a, :],
                        h_sb[:, bt, :],
                        start=(i == 0), stop=stop,
                        tile_position=(0, 32 * t),
                    )

        def sigmoid_store(g):
            """Dense sigmoid + store for 128-attr group g."""
            o_out = osb.tile([128, NBT, BT], mybir.dt.float32, name="osb")
            nc.scalar.activation(
                out=o_out[:], in_=o_ps[0][:],
                func=mybir.ActivationFunctionType.Sigmoid,
                bias=b2_sb[:, g:g + 1], scale=1.0)
            nc.sync.dma_start(
                out=ot[g * 128:(g + 1) * 128, :].rearrange(
                    "p (n b) -> p n b", n=NBT),
                in_=o_out[:])

        x_tile = [None]
        pend = []           # (a, h_sb) relu'd attrs not yet MM2'd
        sig_g = None        # group whose sigmoid/store is deferred
        next_quad = 0
        for p in range(NPAIR):
            blk, off = divmod(p, XBLK)
            if off == 0:
                x_tile[0] = xpool.tile([128, min(XBLK, NPAIR - p), BS],
                                       mybir.dt.bfloat16, name="xt")
                nc.sync.dma_start(
                    out=x_tile[0][:],
                    in_=xs[:, p:p + x_tile[0].shape[1], :])
            # Emit the deferred sigmoid and the oldest complete quad's MM2s
            # BEFORE this pair's MM1s: the MM1 that waits on a PSUM slot
            # (3-deep h pool) must not block ready MM2 work behind it on
            # the strict-FIFO PE queue.
            if len(pend) >= 6:
                if sig_g is not None:
                    sigmoid_store(sig_g)
                    sig_g = None
                mm2_quad(next_quad, pend[:4])
                if (next_quad % 32 == 31) or next_quad == NQUAD - 1:
                    sig_g = next_quad // 32
                next_quad += 1
                pend = pend[4:]
            h_pss = [hps.tile([128, NBT, BT], mybir.dt.float32, name="hps"),
                     hps.tile([128, NBT, BT], mybir.dt.float32, name="hps")]
            # j-outer: j0's two matmuls stream while j1 may still wait on
            # its slot's relu (WAR); bt-outer would trap j0's bt1 behind
            # j1's stall.
            for j in range(2):
                for bt in range(NBT):
                    nc.tensor.matmul(
                        h_pss[j][:, bt, :],
                        w1_sb[j * 64:(j + 1) * 64, p, :],
                        x_tile[0][j * 64:(j + 1) * 64, off,
                                  bass.ds(bt * BT, BT)],
                        start=True, stop=True,
                        tile_position=(j * 64, 0),
                    )
            for j in range(2):
                a = 2 * p + j
                h_sb = hsb.tile([HID, NBT, BT], mybir.dt.bfloat16,
                                name="hsb")
                if use_act[a]:
                    nc.scalar.activation(
                        out=h_sb[:], in_=h_pss[j][:],
                        func=mybir.ActivationFunctionType.Relu,
                        bias=b1_sb[:, a:a + 1], scale=1.0)
                else:
                    nc.vector.tensor_scalar(
                        out=h_sb[:], in0=h_pss[j][:],
                        scalar1=b1_sb[:, a:a + 1], scalar2=0.0,
                        op0=mybir.AluOpType.add,
                        op1=mybir.AluOpType.max)
                pend.append((a, h_sb))
        while pend:
            if sig_g is not None:
                sigmoid_store(sig_g)
                sig_g = None
            mm2_quad(next_quad, pend[:4])
            if (next_quad % 32 == 31) or next_quad == NQUAD - 1:
                sig_g = next_quad // 32
            next_quad += 1
            pend = pend[4:]
        sigmoid_store(sig_g)


def _install_ntff_hook():
    """Register the axon NTFF profile hook (normally provided by the agent
    image's antenv.axon_hooks). Needed only for trace=True runs."""
    import sys as _sys, types as _types, ctypes, contextlib

    if "antenv.axon_hooks" not in _sys.modules:
        mod = _types.ModuleType("antenv.axon_hooks")
        _h = [None]
        mod.set_axon_ntff_profile_hook = lambda h: _h.__setitem__(0, h)
        mod.get_axon_ntff_profile_hook = lambda: _h[0]
        _sys.modules["antenv.axon_hooks"] = mod
        try:
            import antenv
            antenv.axon_hooks = mod
        except ImportError:
            pass
    mod = _sys.modules["antenv.axon_hooks"]
    if mod.get_axon_ntff_profile_hook() is not None:
        return

    lib = ctypes.CDLL("/opt/axon/libaxon_pjrt.so")
    lib.axon_start_nrt_profile.argtypes = [
        ctypes.POINTER(ctypes.c_int64), ctypes.c_size_t]
    lib.axon_start_nrt_profile.restype = ctypes.c_int64
    lib.axon_stop_nrt_profile.argtypes = [ctypes.c_char_p]
    lib.axon_stop_nrt_profile.restype = ctypes.c_int64

    @contextlib.contextmanager
    def _hook(output_dir, device_ids):
        import jax
        jax.devices()
        if device_ids:
            ids = (ctypes.c_int64 * len(device_ids))(*device_ids)
            rc = lib.axon_start_nrt_profile(ids, len(device_ids))
        else:
            rc = lib.axon_start_nrt_profile(None, 0)
        if rc != 0:
            raise RuntimeError(f"axon_start_nrt_profile rc={rc}")
        try:
            yield
        finally:
            n = lib.axon_stop_nrt_profile(str(output_dir).encode())
            print(f"ntff profile: {n} file(s) -> {output_dir}")

    mod.set_axon_ntff_profile_hook(_hook)
    # artifact upload needs a bucket; stub it out for local profiling
    bass_utils.upload_artifacts = lambda tmpdir: f"local://{tmpdir}"


def kernel(x, W1, b1, W2, b2, trace=False):
    if "nc" not in _cached:
        _cached["nc"] = _build_nc()
    nc = _cached["nc"]
    if trace:
        try:
            _install_ntff_hook()
        except Exception as e:
            print("ntff hook install failed:", e)
            trace = False

    xt = np.ascontiguousarray(
        x.reshape(B, A * LAT).astype(ml_dtypes.bfloat16).T)     # [19968, 8192]
    w1h = np.ascontiguousarray(
        W1.reshape(NPAIR, 128, 128).transpose(1, 0, 2)).astype(
            ml_dtypes.bfloat16)                                  # [128,156,128]
    # one-hot padded W2: column (a%128)//4 of slot a holds W2[a]
    w2h = np.zeros((HID, A, 32), np.float32)
    cols = (np.arange(A) % 128) // 4
    w2h[:, np.arange(A), cols] = W2.reshape(A, HID).T
    w2h = w2h.astype(ml_dtypes.bfloat16)
    b1h = np.ascontiguousarray(b1.T).astype(np.float32)          # [128, 312]
    # b2 permuted to dense-o partition order
    b2h = np.zeros((128, NGRP), np.float32)
    for g in range(NGRP):
        m = np.arange(min(128, A - g * 128))
        b2h[32 * (m % 4) + m // 4, g] = b2.reshape(A)[g * 128 + m]

    in_maps = []
    for c in range(NCORES):
        xc = xt[:, c * BS:(c + 1) * BS]                          # [19968,1024]
        xsh = np.ascontiguousarray(
            xc.reshape(NPAIR, 128, BS).transpose(1, 0, 2))       # [128,156,1024]
        in_maps.append({
            "xs": xsh, "w1": w1h, "w2": w2h, "b1": b1h, "b2": b2h,
        })

    res = bass_utils.run_bass_kernel_spmd(
        nc, in_maps, core_ids=list(range(NCORES)), trace=trace)
    _cached["last_results"] = res

    # undo the dense-o partition permutation: attr a = g*128+m lives at
    # device row g*128 + 32*(m%4) + m//4
    aa = np.arange(A)
    g, m = aa // 128, aa % 128
    rows = g * 128 + 32 * (m % 4) + m // 4
    out = np.empty((B, A), np.float32)
    for c in range(NCORES):
        out[c * BS:(c + 1) * BS, :] = res.results[c]["ot"][rows, :].T
    return out


# revision 10
# speedup vs baseline: 1.1061x; 1.0221x over previous
"""Trainium2 Bass kernel for per-attribute MLP decoder (nn_AttrDecoder).

Computes, for each attribute a (A=312 independent blocks):
    h = relu(x[:, a*64:(a+1)*64] @ W1[a] + b1[a])      # [B, 128]
    o[:, a] = sigmoid(h @ W2[a] + b2[a])               # [B, 1]

v2 design notes (vs the 292us v1 baseline):
  - The binding constraint is the PSUM->SBUF relu pass over h (40.9M
    elems/core): only ACT and DVE can read PSUM, at ~1 elem/cycle for
    fp32 sources, so that stage floors at ~170us across both engines.
    Everything else is arranged to stay under that and off those engines.
  - v1 burned ~86us of ACT on sigmoids over [128,1024] tiles with only
    4 live partitions (one per PE column-group).  v2 packs o densely:
    MM2 for attr a uses an M=32 one-hot stationary (W2[a] in column
    (a%128)//4, zeros elsewhere) accumulated onto a shared PSUM bank at
    tile_position (0, 32*(a%4)), so 128 attrs land on 128 distinct
    partitions.  Sigmoid+b2 then runs once per 128-attr group (3 ops
    total, ~3us) with b2 as a per-partition bias, and the store is one
    dense DMA per group (host undoes the partition permutation).
  - x DMAs are 3 MB each (12 pairs, 24 KB contiguous per partition)
    instead of 156x 256 KB: near line-rate HBM streaming.
  - relu work is split ACT:DVE ~169:143 to balance their effective
    per-op rates (1.01us vs 1.19us for FD=1024).
"""

import numpy as np
import ml_dtypes

import concourse.bass as bass
import concourse.tile as tile
from concourse import mybir
from concourse import bass_utils

A = 312
LAT = 64
HID = 128
B = 8192
NCORES = 8
BS = B // NCORES          # 1024 batch rows per core
NPAIR = A // 2            # 156
NQUAD = A // 4            # 78
BT = 512                  # batch tile (one PSUM bank of fp32)
NBT = BS // BT            # 2
XBLK = 12                 # pairs per x DMA (3 MB, 24 KB/partition)
NGRP = (A + 127) // 128   # 3 output groups (128 attrs dense per group)
N_ACT = 167               # relu ops assigned to ScalarE (rest on DVE)

_cached = {}


def _legalize_waits(nc, max_waits=1):
    """Walrus in this toolchain encodes at most one sync-wait per instruction.
    Hoist extra waits onto standalone EventSemaphore instructions placed just
    before the owner on the same engine queue (queue order preserves the
    happens-before)."""
    nsplit = 0
    for bb in nc.m.functions[0].blocks:
        new_insts = []
        changed = False
        for inst in bb.instructions:
            si = getattr(inst, "sync_info", None)
            if si is not None and len(si.on_wait) > max_waits:
                waits = list(si.on_wait)
                for k, w in enumerate(waits[:-max_waits]):
                    es = mybir.InstEventSemaphore(name=f"{inst.name}-hw{k}")
                    es.engine = inst.engine
                    es.opcode = "EventSemaphore"
                    es.sync_info = mybir.SyncInfo(on_wait=[w], on_update=[])
                    new_insts.append(es)
                    nsplit += 1
                inst.sync_info = mybir.SyncInfo(
                    on_wait=waits[-max_waits:], on_update=list(si.on_update))
                changed = True
            new_insts.append(inst)
        if changed:
            bb.instructions = new_insts
    return nsplit


def _build_nc():
    nc = bass.Bass("TRN2", target_bir_lowering=False, debug=False,
                   num_devices=NCORES)
    # xs[r, q, :] = x^T[q*128 + r, :] so each pair q is one partition-dim
    # slice and per-partition reads are contiguous across consecutive pairs.
    xs = nc.dram_tensor("xs", [128, NPAIR, BS], mybir.dt.bfloat16,
                        kind="ExternalInput").ap()
    w1 = nc.dram_tensor("w1", [128, NPAIR, 128], mybir.dt.bfloat16,
                        kind="ExternalInput").ap()
    # One-hot padded W2: w2oh[:, a, i] = W2[a] if i == (a%128)//4 else 0.
    w2 = nc.dram_tensor("w2", [HID, A, 32], mybir.dt.bfloat16,
                        kind="ExternalInput").ap()
    b1 = nc.dram_tensor("b1", [HID, A], mybir.dt.float32,
                        kind="ExternalInput").ap()
    # b2 permuted to the dense-o partition order: b2g[p, g] = b2[g*128+m]
    # with p = 32*(m%4) + m//4.
    b2 = nc.dram_tensor("b2", [128, NGRP], mybir.dt.float32,
                        kind="ExternalInput").ap()
    ot = nc.dram_tensor("ot", [NGRP * 128, BS], mybir.dt.float32,
                        kind="ExternalOutput").ap()

    with tile.TileContext(nc, trace_sim=False) as tc:
        _body(tc, xs, w1, w2, b1, b2, ot)
    _legalize_waits(nc)
    return nc


def _body(tc, xs, w1, w2, b1, b2, ot):
    nc = tc.nc
    from contextlib import ExitStack
    with ExitStack() as ctx:
        singles = ctx.enter_context(tc.tile_pool(name="singles", bufs=1))
        xpool = ctx.enter_context(tc.tile_pool(name="x", bufs=2))
        hsb = ctx.enter_context(tc.tile_pool(name="hsb", bufs=10))
        osb = ctx.enter_context(tc.tile_pool(name="osb", bufs=2))
        hps = ctx.enter_context(
            tc.tile_pool(name="hps", bufs=3, space=bass.MemorySpace.PSUM))
        ops = ctx.enter_context(
            tc.tile_pool(name="ops", bufs=1, space=bass.MemorySpace.PSUM))

        b1_sb = singles.tile([HID, A], mybir.dt.float32)
        w2_sb = singles.tile([HID, A, 32], mybir.dt.bfloat16)
        b2_sb = singles.tile([128, NGRP], mybir.dt.float32)
        w1_sb = singles.tile([128, NPAIR, 128], mybir.dt.bfloat16)

        # Resident weights: ship what the first pairs need on the fast sync
        # HWDGE queue, bulk follows on the gpsimd SWDGE queue in growing
        # chunks (issue overhead ~1us per SWDGE dma).
        nc.sync.dma_start(w1_sb[:, 0:8, :], w1[:, 0:8, :])
        nc.gpsimd.dma_start(b1_sb[:, 0:64], b1[:, 0:64])
        nc.gpsimd.dma_start(w2_sb[:, 0:32, :], w2[:, 0:32, :])
        nc.gpsimd.dma_start(b2_sb[:], b2[:])
        nc.gpsimd.dma_start(w1_sb[:, 8:20, :], w1[:, 8:20, :])
        nc.gpsimd.dma_start(b1_sb[:, 64:A], b1[:, 64:A])
        nc.gpsimd.dma_start(w2_sb[:, 32:128, :], w2[:, 32:128, :])
        for c, ce in [(20, 36), (36, 60), (60, 90), (90, 124), (124, 156)]:
            nc.gpsimd.dma_start(w1_sb[:, c:ce, :], w1[:, c:ce, :])
        nc.gpsimd.dma_start(w2_sb[:, 128:A, :], w2[:, 128:A, :])

        # relu engine assignment: strict per-pair alternation (even attr on
        # ScalarE, odd on DVE) so a pair's two PSUM slots free at the same
        # time -> the next pair's two MM1s become ready together and the
        # scheduler keeps their row-tiled streams concurrent.  A "balanced"
        # 167:145 split measured WORSE overall: it staggers slot release,
        # the scheduler interleaves MM2s between the j0/j1 chains, and MM1
        # loses its 2x row-group concurrency (PE busy 209us -> pacer).
        use_act = [a % 2 == 0 for a in range(A)]

        o_ps = [None]       # current group's dense o accumulator

        def mm2_quad(k, quad):
            """Quad k = attrs 4k..4k+3 -> col-groups 0..3 (concurrent
            streams), accumulating into o_ps at chain position i=k%32."""
            i = k % 32
            stop = (i == 31) or (k == NQUAD - 1)
            if i == 0:
                o_ps[0] = ops.tile([128, NBT, BT], mybir.dt.float32,
                                   name="o_dense")
            for bt in range(NBT):
                for t, (a, h_sb) in enumerate(quad):
                    # start=True clears has_written for the written region
                    # only (verified on HW: bank-wide-clear would break the
                    # other col-groups' chains), so each chain opens with it.
                    nc.tensor.matmul(
                        o_ps[0][32 * t:32 * t + 32, bt, :],
                        w2_sb[:, a, :],
                        h_sb[:, bt, :],
                        start=(i == 0), stop=stop,
                        tile_position=(0, 32 * t),
                    )

        def sigmoid_store(g):
            """Dense sigmoid + store for 128-attr group g."""
            o_out = osb.tile([128, NBT, BT], mybir.dt.float32, name="osb")
            nc.scalar.activation(
                out=o_out[:], in_=o_ps[0][:],
                func=mybir.ActivationFunctionType.Sigmoid,
                bias=b2_sb[:, g:g + 1], scale=1.0)
            nc.sync.dma_start(
                out=ot[g * 128:(g + 1) * 128, :].rearrange(
                    "p (n b) -> p n b", n=NBT),
                in_=o_out[:])

        x_tile = [None]
        pend = []           # (a, h_sb) relu'd attrs not yet MM2'd
        sig_g = None        # group whose sigmoid/store is deferred
        next_quad = 0
        for p in range(NPAIR):
            blk, off = divmod(p, XBLK)
            if off == 0:
                x_tile[0] = xpool.tile([128, min(XBLK, NPAIR - p), BS],
                                       mybir.dt.bfloat16, name="xt")
                nc.sync.dma_start(
                    out=x_tile[0][:],
                    in_=xs[:, p:p + x_tile[0].shape[1], :])
            # Emit the deferred sigmoid and the oldest complete quad's MM2s
            # BEFORE this pair's MM1s: the MM1 that waits on a PSUM slot
            # (3-deep h pool) must not block ready MM2 work behind it on
            # the strict-FIFO PE queue.
            if len(pend) >= 6:
                if sig_g is not None:
                    sigmoid_store(sig_g)
                    sig_g = None
                mm2_quad(next_quad, pend[:4])
                if (next_quad % 32 == 31) or next_quad == NQUAD - 1:
                    sig_g = next_quad // 32
                next_quad += 1
                pend = pend[4:]
            h_pss = [hps.tile([128, NBT, BT], mybir.dt.float32, name="hps"),
                     hps.tile([128, NBT, BT], mybir.dt.float32, name="hps")]
            # bt-outer / j-inner: with per-pair engine alternation both
            # slots free together, so j0/j1 issue back-to-back and stream
            # concurrently on disjoint PE row groups.
            for bt in range(NBT):
                for j in range(2):
                    nc.tensor.matmul(
                        h_pss[j][:, bt, :],
                        w1_sb[j * 64:(j + 1) * 64, p, :],
                        x_tile[0][j * 64:(j + 1) * 64, off,
                                  bass.ds(bt * BT, BT)],
                        start=True, stop=True,
                        tile_position=(j * 64, 0),
                    )
            for j in range(2):
                a = 2 * p + j
                h_sb = hsb.tile([HID, NBT, BT], mybir.dt.bfloat16,
                                name="hsb")
                if use_act[a]:
                    nc.scalar.activation(
                        out=h_sb[:], in_=h_pss[j][:],
                        func=mybir.ActivationFunctionType.Relu,
                        bias=b1_sb[:, a:a + 1], scale=1.0)
                else:
                    nc.vector.tensor_scalar(
                        out=h_sb[:], in0=h_pss[j][:],
                        scalar1=b1_sb[:, a:a + 1], scalar2=0.0,
                        op0=mybir.AluOpType.add,
                        op1=mybir.AluOpType.max)
                pend.append((a, h_sb))
        while pend:
            if sig_g is not None:
                sigmoid_store(sig_g)
                sig_g = None
            mm2_quad(next_quad, pend[:4])
            if (next_quad % 32 == 31) or next_quad == NQUAD - 1:
                sig_g = next_quad // 32
            next_quad += 1
            pend = pend[4:]
        sigmoid_store(sig_g)


def _install_ntff_hook():
    """Register the axon NTFF profile hook (normally provided by the agent
    image's antenv.axon_hooks). Needed only for trace=True runs."""
    import sys as _sys, types as _types, ctypes, contextlib

    if "antenv.axon_hooks" not in _sys.modules:
        mod = _types.ModuleType("antenv.axon_hooks")
        _h = [None]
        mod.set_axon_ntff_profile_hook = lambda h: _h.__setitem__(0, h)
        mod.get_axon_ntff_profile_hook = lambda: _h[0]
        _sys.modules["antenv.axon_hooks"] = mod
        try:
            import antenv
            antenv.axon_hooks = mod
        except ImportError:
            pass
    mod = _sys.modules["antenv.axon_hooks"]
    if mod.get_axon_ntff_profile_hook() is not None:
        return

    lib = ctypes.CDLL("/opt/axon/libaxon_pjrt.so")
    lib.axon_start_nrt_profile.argtypes = [
        ctypes.POINTER(ctypes.c_int64), ctypes.c_size_t]
    lib.axon_start_nrt_profile.restype = ctypes.c_int64
    lib.axon_stop_nrt_profile.argtypes = [ctypes.c_char_p]
    lib.axon_stop_nrt_profile.restype = ctypes.c_int64

    @contextlib.contextmanager
    def _hook(output_dir, device_ids):
        import jax
        jax.devices()
        if device_ids:
            ids = (ctypes.c_int64 * len(device_ids))(*device_ids)
            rc = lib.axon_start_nrt_profile(ids, len(device_ids))
        else:
            rc = lib.axon_start_nrt_profile(None, 0)
        if rc != 0:
            raise RuntimeError(f"axon_start_nrt_profile rc={rc}")
        try:
            yield
        finally:
            n = lib.axon_stop_nrt_profile(str(output_dir).encode())
            print(f"ntff profile: {n} file(s) -> {output_dir}")

    mod.set_axon_ntff_profile_hook(_hook)
    # artifact upload needs a bucket; stub it out for local profiling
    bass_utils.upload_artifacts = lambda tmpdir: f"local://{tmpdir}"


def kernel(x, W1, b1, W2, b2, trace=False):
    if "nc" not in _cached:
        _cached["nc"] = _build_nc()
    nc = _cached["nc"]
    if trace:
        try:
            _install_ntff_hook()
        except Exception as e:
            print("ntff hook install failed:", e)
            trace = False

    xt = np.ascontiguousarray(
        x.reshape(B, A * LAT).astype(ml_dtypes.bfloat16).T)     # [19968, 8192]
    w1h = np.ascontiguousarray(
        W1.reshape(NPAIR, 128, 128).transpose(1, 0, 2)).astype(
            ml_dtypes.bfloat16)                                  # [128,156,128]
    # one-hot padded W2: column (a%128)//4 of slot a holds W2[a]
    w2h = np.zeros((HID, A, 32), np.float32)
    cols = (np.arange(A) % 128) // 4
    w2h[:, np.arange(A), cols] = W2.reshape(A, HID).T
    w2h = w2h.astype(ml_dtypes.bfloat16)
    b1h = np.ascontiguousarray(b1.T).astype(np.float32)          # [128, 312]
    # b2 permuted to dense-o partition order
    b2h = np.zeros((128, NGRP), np.float32)
    for g in range(NGRP):
        m = np.arange(min(128, A - g * 128))
        b2h[32 * (m % 4) + m // 4, g] = b2.reshape(A)[g * 128 + m]

    in_maps = []
    for c in range(NCORES):
        xc = xt[:, c * BS:(c + 1) * BS]                          # [19968,1024]
        xsh = np.ascontiguousarray(
            xc.reshape(NPAIR, 128, BS).transpose(1, 0, 2))       # [128,156,1024]
        in_maps.append({
            "xs": xsh, "w1": w1h, "w2": w2h, "b1": b1h, "b2": b2h,
        })

    res = bass_utils.run_bass_kernel_spmd(
        nc, in_maps, core_ids=list(range(NCORES)), trace=trace)
    _cached["last_results"] = res

    # undo the dense-o partition permutation: attr a = g*128+m lives at
    # device row g*128 + 32*(m%4) + m//4
    aa = np.arange(A)
    g, m = aa // 128, aa % 128
    rows = g * 128 + 32 * (m % 4) + m // 4
    out = np.empty((B, A), np.float32)
    for c in range(NCORES):
        out[c * BS:(c + 1) * BS, :] = res.results[c]["ot"][rows, :].T
    return out
